# revision 13
# baseline (speedup 1.0000x reference)
"""Causal multi-head attention (B=4, S=2048, D=1024, H=16, Dh=64) on 8 TRN2
NeuronCores.

Sharding: core c -> batch b = c//2, head-group g = c%2 (8 heads each).
W_q/W_k/W_v column-parallel (512 cols per core), W_o row-parallel (512 rows).
Each core computes a partial O^T [1024, 2048] for its batch; host sums the
two head-group partials per batch and transposes back to [S, D].

Per-core pipeline (streamed over 4 s-blocks of 512 rows, causal):
  x rows -> PE-transpose -> x^T block
  Q^T, K^T = W^T @ x^T (heads packed two-per-128-partition tile)
  V = x @ W_v, stored as [t, 65] tiles per (t-tile, head) with a ones column
  per head: S^T tile = K_tile @ Q^T (K=64), exp(score/8) on ACT,
            causal mask via gpsimd affine_select on diagonal tiles,
            out' = [V|1]^T @ P^T accumulated in PSUM ([65, 512]; row 64 = sums)
  normalize: recip(sums), broadcast via ones-matmul, DVE mul
  O^T partial = W_o^T @ outcat^T -> DMA to DRAM
"""

import os
from contextlib import ExitStack

import numpy as np

import concourse.bass as bass
import concourse.mybir as mybir
import concourse.tile as tile
from concourse import bacc
from concourse.bass_utils import run_bass_kernel_spmd
from concourse.masks import make_identity

F32 = mybir.dt.float32

S = 2048          # sequence length
D = 1024          # model dim
HL = 8            # local heads per core
DH = 64           # head dim
CL = HL * DH      # local cols (512)
SBS = 512         # s-block size
NSB = S // SBS    # 4 s-blocks
TS = 128          # tile size (partitions)
NTT = S // TS     # 16 t-tiles
N_CORES = 8

# Matmul compute dtype: float32r runs the PE at 4x the fp32 rate (1 cycle/row
# for N>=256) with ~tf32 precision. Set BASS_MM_F32=1 to force full fp32.
MM_DTYPE = F32 if os.environ.get("BASS_MM_F32") else mybir.dt.float32r


def _mc(ap):
    """View an fp32 AP as the matmul compute dtype (bit-identical storage)."""
    if MM_DTYPE == F32:
        return ap
    return ap.bitcast(MM_DTYPE)


def build_program(mm_dtype=None):
    global MM_DTYPE
    if mm_dtype is not None:
        MM_DTYPE = mm_dtype

    nc = bacc.Bacc(
        "TRN2", target_bir_lowering=False, debug=False, num_devices=N_CORES
    )
    x_d = nc.dram_tensor("x", [S, D], F32, kind="ExternalInput").ap()
    wq_d = nc.dram_tensor("wq", [D, CL], F32, kind="ExternalInput").ap()
    wk_d = nc.dram_tensor("wk", [D, CL], F32, kind="ExternalInput").ap()
    wv_d = nc.dram_tensor("wv", [D, CL], F32, kind="ExternalInput").ap()
    wo_d = nc.dram_tensor("wo", [CL, D], F32, kind="ExternalInput").ap()
    out_d = nc.dram_tensor("out", [D, S], F32, kind="ExternalOutput").ap()

    EXP = mybir.ActivationFunctionType.Exp
    SCALE = 1.0 / 8.0  # 1/sqrt(DH)

    with (
        tile.TileContext(nc) as tc,
        ExitStack() as ctx,
        nc.allow_low_precision(reason="float32r matmul compute dtype"),
    ):
        const = ctx.enter_context(tc.tile_pool(name="const", bufs=1))
        wpool = ctx.enter_context(tc.tile_pool(name="w", bufs=1))
        ktp = ctx.enter_context(tc.tile_pool(name="kt", bufs=1))
        vp = ctx.enter_context(tc.tile_pool(name="v", bufs=1))
        qtp = ctx.enter_context(tc.tile_pool(name="qt", bufs=1))
        xtp = ctx.enter_context(tc.tile_pool(name="xt", bufs=1))
        xsp = ctx.enter_context(tc.tile_pool(name="xs", bufs=2))
        ptp = ctx.enter_context(tc.tile_pool(name="pt", bufs=5))
        ocp = ctx.enter_context(tc.tile_pool(name="oc", bufs=2))
        stp = ctx.enter_context(tc.tile_pool(name="st", bufs=2))
        mp = ctx.enter_context(tc.tile_pool(name="misc", bufs=2))
        psA = ctx.enter_context(tc.tile_pool(name="psA", bufs=6, space="PSUM"))
        psO = ctx.enter_context(tc.tile_pool(name="psO", bufs=2, space="PSUM"))

        ident = const.tile([TS, TS], F32)
        make_identity(nc, ident[:])
        ones64 = const.tile([1, DH], F32)
        nc.gpsimd.memset(ones64[:], 1.0)

        # Weights. wq/wk/wv: [D, CL] -> [128, 8 dtiles x 512]; d = 128*j + p.
        # wo: [CL, D] -> [128, 4 ctiles x 1024]; c = 128*j + p.
        # Walrus requires every writer of an fp32r-matmul input to round to
        # fp32r, and DMA cannot round -- so stage DMA chunks through the xs
        # pool and make a rounding tensor_copy the sole writer of w tiles.
        wq_s = wpool.tile([TS, 8 * CL], F32)
        wk_s = wpool.tile([TS, 8 * CL], F32)
        wv_s = wpool.tile([TS, 8 * CL], F32)
        wo_s = wpool.tile([TS, 4 * D], F32)
        for w_s, w_d in (
            (wq_s, wq_d),
            (wk_s, wk_d),
            (wv_s, wv_d),
            (wo_s, wo_d),
        ):
            src = w_d.rearrange("(j p) c -> p j c", p=TS)  # [128, nj, cols]
            nj = src.shape[1]
            per = nj // 4  # d/c-tiles per 1024-col chunk
            for k in range(4):
                ws = xsp.tile([TS, D], F32, tag="xs", name=f"ws_{k}")
                nc.sync.dma_start(
                    ws[:].rearrange("p (j c) -> p j c", j=per),
                    src[:, k * per : (k + 1) * per, :],
                )
                nc.vector.tensor_copy(
                    _mc(w_s[:, k * D : (k + 1) * D]), ws[:]
                )

        # K^T persistent: [128, 4 ctiles x 2048]; head h -> ctile h//2,
        # partition offset (h%2)*64.  V persistent with ones cols:
        # [128, (16 ttiles x 8 heads) x 65]; [:, :64] = V tile, [:, 64] = 1.
        kt_s = ktp.tile([TS, 4 * S], F32)
        v_s = vp.tile([TS, NTT * HL * 65], F32)
        if MM_DTYPE == F32:
            nc.vector.memset(v_s[:], 1.0)
        else:
            # memset cannot emit fp32r; round 1.0s into the ones columns
            # through an fp32 staging tile.
            vtmp = xsp.tile([TS, NTT * HL], F32, tag="xs")
            nc.vector.memset(vtmp[:], 1.0)
            nc.vector.tensor_copy(
                _mc(v_s[:].rearrange("p (n c) -> p n c", c=65)[:, :, 64]),
                vtmp[:],
            )

        for sb in range(NSB):
            # ---- stage A: load x rows, transpose to x^T block ----
            xt = xtp.tile([TS, 8 * SBS], F32)  # dtile j at cols j*512
            for stl in range(SBS // TS):
                st = sb * (SBS // TS) + stl
                xs = xsp.tile([TS, D], F32)
                nc.sync.dma_start(xs[:], x_d[st * TS : (st + 1) * TS, :])
                for j in range(8):
                    tp = psA.tile([TS, TS], F32, tag="ps")
                    nc.tensor.transpose(
                        tp[:], xs[:, j * TS : (j + 1) * TS], ident[:]
                    )
                    nc.vector.tensor_copy(
                        _mc(xt[:, j * SBS + stl * TS : j * SBS + (stl + 1) * TS]),
                        tp[:],
                    )

            # ---- stage B: projections for this s-block ----
            qt = qtp.tile([TS, 4 * SBS], F32)  # ctile at cols ct*512
            for ct in range(4):
                for w_s, isq in ((wq_s, True), (wk_s, False)):
                    ps = psA.tile([TS, SBS], F32, tag="ps")
                    for j in range(8):
                        nc.tensor.matmul(
                            ps[:],
                            _mc(w_s[:, j * CL + ct * TS : j * CL + (ct + 1) * TS]),
                            _mc(xt[:, j * SBS : (j + 1) * SBS]),
                            start=(j == 0),
                            stop=(j == 7),
                        )
                    if isq:
                        nc.vector.tensor_copy(
                            _mc(qt[:, ct * SBS : (ct + 1) * SBS]), ps[:]
                        )
                    else:
                        nc.vector.tensor_copy(
                            _mc(kt_s[:, ct * S + sb * SBS : ct * S + (sb + 1) * SBS]),
                            ps[:],
                        )
            for stl in range(SBS // TS):
                tt = sb * (SBS // TS) + stl
                ps = psA.tile([TS, SBS], F32, tag="ps")
                for j in range(8):
                    nc.tensor.matmul(
                        ps[:],
                        _mc(xt[:, j * SBS + stl * TS : j * SBS + (stl + 1) * TS]),
                        _mc(wv_s[:, j * CL : (j + 1) * CL]),
                        start=(j == 0),
                        stop=(j == 7),
                    )
                nc.vector.tensor_copy(
                    _mc(v_s[:].rearrange("p (n c) -> p n c", c=65)[
                        :, tt * HL : (tt + 1) * HL, 0:64
                    ]),
                    ps[:].rearrange("p (h e) -> p h e", e=DH),
                )

            # ---- stage C: attention, head pairs (even at partitions 0:64,
            # odd at 64:128 -> independent PE row groups + 2 chains in flight)
            oc = ocp.tile([TS, 4 * SBS], F32)  # outcat^T: ctile at cols ct*512
            ntt = (sb + 1) * (SBS // TS)
            for hp in range(HL // 2):
                ct = hp
                heads = (2 * hp, 2 * hp + 1)
                pos = [
                    psO.tile([TS, SBS], F32, tag="po", name=f"po_{sb}_{hp}_{i}")
                    for i in range(2)
                ]
                for tt in range(ntt):
                    pts = []
                    for i, h in enumerate(heads):
                        poff = (h % 2) * DH
                        ps = psA.tile([TS, SBS], F32, tag="ps")
                        nc.tensor.matmul(
                            ps[:],
                            _mc(
                                kt_s[
                                    poff : poff + DH,
                                    ct * S + tt * TS : ct * S + (tt + 1) * TS,
                                ]
                            ),
                            _mc(qt[poff : poff + DH, ct * SBS : (ct + 1) * SBS]),
                            start=True,
                            stop=True,
                        )
                        pt = ptp.tile([TS, SBS], F32)
                        nc.scalar.activation(_mc(pt[:]), ps[:], EXP, scale=SCALE)
                        if tt >= sb * (SBS // TS):
                            k = tt - sb * (SBS // TS)
                            # keep where s_local >= t_local + 128*k
                            nc.gpsimd.affine_select(
                                out=_mc(pt[:]),
                                in_=_mc(pt[:]),
                                compare_op=mybir.AluOpType.is_ge,
                                fill=0.0,
                                base=-(TS * k),
                                channel_multiplier=-1,
                                pattern=[[1, SBS]],
                            )
                        pts.append(pt)
                    for i, h in enumerate(heads):
                        nc.tensor.matmul(
                            pos[i][0:65, :],
                            _mc(
                                v_s[
                                    :, (tt * HL + h) * 65 : (tt * HL + h + 1) * 65
                                ]
                            ),
                            _mc(pts[i][:]),
                            start=(tt == 0),
                            stop=(tt == ntt - 1),
                        )
                # normalize: out[d, s] = po[d, s] * (1 / po[64, s])
                for i, h in enumerate(heads):
                    po = pos[i]
                    poff = (h % 2) * DH
                    pn = stp.tile([65, SBS], F32, tag="pn")
                    nc.vector.tensor_copy(pn[:], po[0:65, :])
                    rs = mp.tile([1, SBS], F32)
                    nc.vector.reciprocal(rs[:], pn[64:65, :])
                    bc = psA.tile([DH, SBS], F32, tag="ps", name=f"bc_{sb}_{hp}_{i}")
                    nc.tensor.matmul(
                        bc[:], ones64[:], rs[:], start=True, stop=True
                    )
                    if poff == 0:
                        nc.vector.tensor_mul(
                            _mc(oc[0:DH, ct * SBS : (ct + 1) * SBS]),
                            pn[0:DH, :],
                            bc[:],
                        )
                    else:
                        stg = stp.tile([DH, SBS], F32)
                        nc.vector.tensor_mul(_mc(stg[:]), pn[0:DH, :], bc[:])
                        # cross-partition move 0..63 -> 64..127 via DMA
                        nc.sync.dma_start(
                            oc[DH : 2 * DH, ct * SBS : (ct + 1) * SBS], stg[:]
                        )

            # ---- stage D: output projection ----
            for mt in range(8):
                ps = psA.tile([TS, SBS], F32, tag="ps")
                for j in range(4):
                    nc.tensor.matmul(
                        ps[:],
                        _mc(wo_s[:, j * D + mt * TS : j * D + (mt + 1) * TS]),
                        _mc(oc[:, j * SBS : (j + 1) * SBS]),
                        start=(j == 0),
                        stop=(j == 3),
                    )
                ob = stp.tile([TS, SBS], F32, tag="ob")
                nc.vector.tensor_copy(ob[:], ps[:])
                nc.sync.dma_start(
                    out_d[mt * TS : (mt + 1) * TS, sb * SBS : (sb + 1) * SBS],
                    ob[:],
                )

    nc.compile()
    return nc


_prog_cache = {}


def _get_program():
    key = MM_DTYPE
    if key not in _prog_cache:
        _prog_cache[key] = build_program()
    return _prog_cache[key]


def make_in_maps(inputs):
    x = np.asarray(inputs["x"], np.float32)
    wq = np.asarray(inputs["W_q"], np.float32)
    wk = np.asarray(inputs["W_k"], np.float32)
    wv = np.asarray(inputs["W_v"], np.float32)
    wo = np.asarray(inputs["W_o"], np.float32)
    in_maps = []
    for c in range(N_CORES):
        b, g = c // 2, c % 2
        cs = slice(g * CL, (g + 1) * CL)
        in_maps.append(
            {
                "x": np.ascontiguousarray(x[b]),
                "wq": np.ascontiguousarray(wq[:, cs]),
                "wk": np.ascontiguousarray(wk[:, cs]),
                "wv": np.ascontiguousarray(wv[:, cs]),
                "wo": np.ascontiguousarray(wo[cs, :]),
            }
        )
    return in_maps


def run(inputs, trace=False, **kwargs):
    nc = _get_program()
    res = run_bass_kernel_spmd(
        nc, make_in_maps(inputs), core_ids=list(range(N_CORES)),
        trace=trace, **kwargs
    )
    outs = [res.results[c]["out"] for c in range(N_CORES)]
    full = np.stack(
        [(outs[2 * b] + outs[2 * b + 1]).T for b in range(4)]
    ).astype(np.float32)
    return full, res


def kernel(**inputs) -> np.ndarray:
    out, _ = run(inputs)
    return out
